# revision 7
# baseline (speedup 1.0000x reference)
"""GQA (B=2, S=2048, d_model=2048, 16 Q heads / 4 KV groups) + output projection.

Sharding: 8 cores, core c <-> (b = c//4, g = c%4). Each core computes full
attention for the 4 query heads of KV group g of batch b, then multiplies its
512-feature slice of the concatenated head outputs with the matching 512 rows
of Wc^T, producing a partial [S, d_model] projection (bf16). Host sums the 4
partials per batch element and adds the bias.

On-core layout: everything transposed.
  scoresT[t, s] = kT.T @ qT            (lhsT = kT tile [d,128t], rhs = qT [d,512s])
  expT = exp(scoresT / sqrt(128))      (ACT, fused scale, bf16 out; no max
                                        subtraction: scores ~ N(0,1))
  sums  = partition_all_reduce(H-tree(expT))
                                       (DVE bf16 4-op halving tree + one GPSIMD
                                        partition all-reduce -- NO PE matmuls
                                        for the softmax denominator)
  uT[hd, s]   = v.T @ expT             (PE, bf16 operands, accumulated over 16 t)
  attnT = uT * recip(sums)             (DVE recip + mult, deferred one combo)
  out[s, o]   = attnT.T @ wT           (PE fp32r, contraction over 512 features;
                                        bias added on host)

Scheduling: per-iteration k the PE stream interleaves, per 2-t-tile step,
QK(k) pairs + PV(k-1) pairs + 2 proj matmuls of group (k-1)//4-1 (projection
spread into the following group's combos keeps PE cadence ~10.2us/combo above
the ACT exp rate ~10.7us/combo). The k-1 normalize (divide) is emitted at the
END of iteration k so the Pool all-reduce latency (~4.3us) never head-of-line
blocks the in-order DVE queue.
"""

import math
import sys

sys.path.insert(0, "/opt/trn_rl_repo")

import ml_dtypes
import numpy as np

import concourse.bacc as bacc
import concourse.bass as bass
import concourse.bass_isa as bass_isa
import concourse.mybir as mybir
import concourse.tile as tile
from concourse.bass import ds, ts
from concourse.bass_utils import run_bass_kernel_spmd

F32 = mybir.dt.float32
F32R = mybir.dt.float32r
BF16 = mybir.dt.bfloat16

B = 2
S = 2048
D_MODEL = 2048
N_GROUPS = 4
HEADS_PER_GROUP = 4
HEAD_DIM = 128
P = 128
NT = S // P          # 16 t tiles
SCALE = 1.0 / math.sqrt(HEAD_DIM)

_COMPILED = None


def _build():
    nc = bacc.Bacc(None, target_bir_lowering=False)

    qT_d = nc.dram_tensor("qT", [P, HEADS_PER_GROUP, S], F32, kind="ExternalInput")
    kT_d = nc.dram_tensor("kT", [P, S], F32, kind="ExternalInput")
    v_d = nc.dram_tensor("v", [S, P], BF16, kind="ExternalInput")
    wT_d = nc.dram_tensor("wT", [HEADS_PER_GROUP * P, D_MODEL], F32, kind="ExternalInput")
    out_d = nc.dram_tensor("out", [S, D_MODEL], BF16, kind="ExternalOutput")

    Exp = mybir.ActivationFunctionType.Exp
    add = mybir.AluOpType.add
    mult = mybir.AluOpType.mult

    n_combos = 16

    with tile.TileContext(nc) as tc:
        with (
            tc.tile_pool(name="const", bufs=1) as const_pool,
            tc.tile_pool(name="qt", bufs=3) as qt_pool,
            tc.tile_pool(name="expT", bufs=2) as expT_pool,
            tc.tile_pool(name="tr8", bufs=2) as tr8_pool,
            tc.tile_pool(name="tr4", bufs=2) as tr4_pool,
            tc.tile_pool(name="tr2", bufs=2) as tr2_pool,
            tc.tile_pool(name="acc", bufs=2) as acc_pool,
            tc.tile_pool(name="sums", bufs=2) as sums_pool,
            tc.tile_pool(name="rb", bufs=2) as rb_pool,
            tc.tile_pool(name="attnT", bufs=8) as attnT_pool,
            tc.tile_pool(name="orow", bufs=2) as orow_pool,
            tc.tile_pool(name="qk_ps", bufs=2, space="PSUM") as qk_psum,
            tc.tile_pool(name="pv_ps", bufs=2, space="PSUM") as pv_psum,
            tc.tile_pool(name="po_ps", bufs=2, space="PSUM") as po_psum,
        ):
            # startup: first QK needs kT chunk 0 + qT(0). Trigger from two
            # engines so the descriptor writes overlap.
            kT_chunks = []
            for c in range(4):
                kc = const_pool.tile([P, 512], F32R, tag=f"kT{c}")
                kT_chunks.append(kc)
            qt0 = qt_pool.tile([P, 512], F32R, tag="qT")
            nc.sync.dma_start(qt0[:, 0:256], qT_d[:, 0, ds(0, 256)].bitcast(F32R))
            nc.scalar.dma_start(qt0[:, 256:512], qT_d[:, 0, ds(256, 256)].bitcast(F32R))
            nc.sync.dma_start(kT_chunks[0][:, 0:256], kT_d[:, ds(0, 256)].bitcast(F32R))
            nc.scalar.dma_start(kT_chunks[0][:, 256:512], kT_d[:, ds(256, 256)].bitcast(F32R))
            nc.sync.dma_start(kT_chunks[1][:], kT_d[:, ts(1, 512)].bitcast(F32R))
            nc.scalar.dma_start(kT_chunks[2][:], kT_d[:, ts(2, 512)].bitcast(F32R))
            nc.scalar.dma_start(kT_chunks[3][:], kT_d[:, ts(3, 512)].bitcast(F32R))
            # v (bf16, 512KB) on the gpsimd queue; wT (4MB) behind it --
            # first use of wT is the first proj slice ~45us in.
            v_sb = const_pool.tile([P, NT, P], BF16, tag="v")
            nc.gpsimd.dma_start(v_sb[:], v_d.rearrange("(n p) d -> p n d", p=P))
            wT_sb = const_pool.tile([P, HEADS_PER_GROUP, D_MODEL], F32R, tag="wT")
            nc.gpsimd.dma_start(
                wT_sb[:], wT_d.rearrange("(n p) o -> p n o", p=P).bitcast(F32R)
            )

            qts = {0: qt0}
            ets = {}      # k -> exp tile [P, 16, 512] bf16
            attnT = {}
            pv_tiles = {}
            sums_tiles = {}

            for k in range(n_combos + 5):
                # prefetch next combo's qT one iteration ahead
                kq = k + 1
                if kq < n_combos and kq not in qts:
                    jq, hq = divmod(kq, HEADS_PER_GROUP)
                    qt = qt_pool.tile([P, 512], F32R, tag="qT")
                    nc.sync.dma_start(qt[:], qT_d[:, hq, ts(jq, 512)].bitcast(F32R))
                    qts[kq] = qt

                do_qk = k < n_combos
                do_pv = 1 <= k <= n_combos
                # proj slice: group jp, row-block st, fed by combos of group jp+1
                do_proj = 5 <= k <= n_combos + 4
                if do_proj:
                    jp, stp = divmod(k - 5, 4)
                    jp_orow = orow_pool.tile([P, D_MODEL], BF16, tag="orow")

                if do_pv:
                    pv_ps = pv_psum.tile([P, 512], F32, tag="pv")
                    pv_tiles[k - 1] = pv_ps

                if do_qk:
                    et = expT_pool.tile([P, NT, 512], BF16, tag="expT")
                    ets[k] = et

                for cc in range(8):
                    if do_qk:
                        ps = qk_psum.tile([P, 2, 512], F32, tag="qk")
                        for u in range(2):
                            tt = 2 * cc + u
                            nc.tensor.matmul(
                                ps[:, u, :],
                                kT_chunks[tt // 4][:, ts(tt % 4, P)],
                                qts[k][:],
                                start=True, stop=True,
                            )
                    if do_pv and cc < 4:
                        for u in range(4):
                            tt = 4 * cc + u
                            nc.tensor.matmul(
                                pv_ps[:],
                                v_sb[:, tt, :],
                                ets[k - 1][:, tt, :],
                                start=(tt == 0), stop=(tt == NT - 1),
                            )
                    if do_pv and cc == 4:
                        # normalize combo k-1 mid-iteration: PV(k-1) just
                        # finished, the k-1 all-reduce ran last iteration, and
                        # the proj matmuls that read attnT(k-1) are a full
                        # iteration away -- no DVE head-of-line blocking
                        rb = rb_pool.tile([P, 512], F32, tag="rb")
                        nc.vector.reciprocal_approx_fast(
                            rb[:], sums_tiles[k - 1][:]
                        )
                        at = attnT_pool.tile([P, 512], F32R, tag="attnT")
                        nc.vector.tensor_tensor(
                            at[:], pv_tiles[k - 1][:], rb[:], mult
                        )
                        attnT[k - 1] = at
                    if do_proj and cc >= 4:
                        ob = cc - 4
                        po = po_psum.tile([P, 512], F32, tag="po")
                        for h in range(4):
                            nc.tensor.matmul(
                                po[:],
                                attnT[4 * jp + h][:, ts(stp, P)],
                                wT_sb[:, h, ts(ob, 512)],
                                start=(h == 0), stop=(h == HEADS_PER_GROUP - 1),
                            )
                        if do_qk:
                            nc.vector.tensor_copy(jp_orow[:, ts(ob, 512)], po[:])
                        else:
                            # tail iterations: ACT is exp-free, use it to copy
                            nc.scalar.copy(jp_orow[:, ts(ob, 512)], po[:])
                    if do_qk:
                        nc.scalar.activation(
                            et[:, ds(2 * cc, 2), :], ps[:], Exp, scale=SCALE
                        )

                if do_proj:
                    nc.sync.dma_start(
                        out_d[ds(jp * 512 + stp * P, P), :], jp_orow[:]
                    )

                if do_qk:
                    # softmax denominator: 4-op halving tree (bf16, 2x DVE),
                    # then cross-partition all-reduce on the Pool engine
                    t8 = tr8_pool.tile([P, 8, 512], BF16, tag="tr8")
                    nc.vector.tensor_tensor(
                        t8[:], et[:, 0:8, :], et[:, 8:16, :], add
                    )
                    t4 = tr4_pool.tile([P, 4, 512], BF16, tag="tr4")
                    nc.vector.tensor_tensor(t4[:], t8[:, 0:4, :], t8[:, 4:8, :], add)
                    t2 = tr2_pool.tile([P, 2, 512], BF16, tag="tr2")
                    nc.vector.tensor_tensor(t2[:], t4[:, 0:2, :], t4[:, 2:4, :], add)
                    acc = acc_pool.tile([P, 512], F32, tag="acc")
                    nc.vector.tensor_tensor(acc[:], t2[:, 0, :], t2[:, 1, :], add)
                    sums_bc = sums_pool.tile([P, 512], F32, tag="sums")
                    nc.gpsimd.partition_all_reduce(
                        sums_bc[:], acc[:], channels=P,
                        reduce_op=bass_isa.ReduceOp.add,
                    )
                    sums_tiles[k] = sums_bc


    nc.compile()
    return nc


def _get_nc():
    global _COMPILED
    if _COMPILED is None:
        _COMPILED = _build()
    return _COMPILED


def _shard_inputs(q, k, v, Wc):
    in_maps = []
    for c in range(8):
        b, g = divmod(c, 4)
        qT = np.ascontiguousarray(
            q[b][:, g * 512:(g + 1) * 512].reshape(S, HEADS_PER_GROUP, P).transpose(2, 1, 0)
        )
        kT = np.ascontiguousarray(k[b][:, g * P:(g + 1) * P].T)
        vv = np.ascontiguousarray(v[b][:, g * P:(g + 1) * P]).astype(ml_dtypes.bfloat16)
        wT = np.ascontiguousarray(Wc[:, g * 512:(g + 1) * 512].T)
        in_maps.append({"qT": qT, "kT": kT, "v": vv, "wT": wT})
    return in_maps


def _run(inputs, trace=False):
    q = np.asarray(inputs["q"], dtype=np.float32)
    k = np.asarray(inputs["k"], dtype=np.float32)
    v = np.asarray(inputs["v"], dtype=np.float32)
    Wc = np.asarray(inputs["Wc"], dtype=np.float32)
    bc = np.asarray(inputs["bc"], dtype=np.float32)

    nc = _get_nc()
    in_maps = _shard_inputs(q, k, v, Wc)
    res = run_bass_kernel_spmd(nc, in_maps, list(range(8)), trace=trace)

    out = np.empty((B, S, D_MODEL), dtype=np.float32)
    for b in range(B):
        acc = res.results[4 * b]["out"].astype(np.float32)
        for g in range(1, 4):
            acc = acc + res.results[4 * b + g]["out"].astype(np.float32)
        acc += bc.reshape(1, D_MODEL)
        out[b] = acc
    return out, res


def kernel(**inputs):
    out, _ = _run(inputs, trace=False)
    return out


# revision 8
# speedup vs baseline: 1.0121x; 1.0121x over previous
"""GQA (B=2, S=2048, d_model=2048, 16 Q heads / 4 KV groups) + output projection.

Sharding: 8 cores, core c <-> (b = c//4, g = c%4). Each core computes full
attention for the 4 query heads of KV group g of batch b, then multiplies its
512-feature slice of the concatenated head outputs with the matching 512 rows
of Wc^T, producing a partial [S, d_model] projection (bf16). Host sums the 4
partials per batch element and adds the bias.

On-core layout: everything transposed.
  scoresT[t, s] = kT.T @ qT            (lhsT = kT tile [d,128t], rhs = qT [d,512s])
  expT = exp(scoresT / sqrt(128))      (ACT, fused scale, bf16 out; no max
                                        subtraction: scores ~ N(0,1))
  sums  = partition_all_reduce(chain(expT))
                                       (DVE bf16 incremental chain adds as each
                                        exp chunk lands + one GPSIMD partition
                                        all-reduce -- NO PE matmuls for the
                                        softmax denominator)
  uT[hd, s]   = v.T @ expT             (PE, bf16 operands, accumulated over 16 t)
  attnT = uT * recip(sums)             (DVE recip + mult, lagged ~1 iteration
                                        behind -- the consumers have >=1
                                        iteration of slack)
  out[s, o]   = attnT.T @ wT           (PE fp32r, contraction over 512 features;
                                        bias added on host)

Steady-state iteration k (one combo = head h of group j, 512-wide s-block):
  PE:  8x [2 QK MMs + 2 PV(k-1) MMs] with the 4 proj po-blocks (4 MMs each) of
       slice (jp=(k-5)//4, st=(k-5)%4) placed in the back half. 48 MMs =
       10.24us cadence.
  ACT: 8 exp chunks (8.9us) + the ob3 po copy.
  DVE: 7 chain adds + fold (5.8us) + po copies ob0-2 + recip(k-1) + mult(k-1),
       in that order so nothing waiting on the Pool all-reduce ever
       head-of-line blocks a copy the PE needs.
  Pool: one partition_all_reduce (3.5us), latency fully hidden.
"""

import math
import sys

sys.path.insert(0, "/opt/trn_rl_repo")

import ml_dtypes
import numpy as np

import concourse.bacc as bacc
import concourse.bass as bass
import concourse.bass_isa as bass_isa
import concourse.mybir as mybir
import concourse.tile as tile
from concourse.bass import ds, ts
from concourse.bass_utils import run_bass_kernel_spmd

F32 = mybir.dt.float32
F32R = mybir.dt.float32r
BF16 = mybir.dt.bfloat16

B = 2
S = 2048
D_MODEL = 2048
N_GROUPS = 4
HEADS_PER_GROUP = 4
HEAD_DIM = 128
P = 128
NT = S // P          # 16 t tiles
SCALE = 1.0 / math.sqrt(HEAD_DIM)

_COMPILED = None


def _build():
    nc = bacc.Bacc(None, target_bir_lowering=False)

    qT_d = nc.dram_tensor("qT", [P, HEADS_PER_GROUP, S], F32, kind="ExternalInput")
    kT_d = nc.dram_tensor("kT", [P, S], F32, kind="ExternalInput")
    v_d = nc.dram_tensor("v", [S, P], BF16, kind="ExternalInput")
    wT_d = nc.dram_tensor("wT", [HEADS_PER_GROUP * P, D_MODEL], F32, kind="ExternalInput")
    out_d = nc.dram_tensor("out", [S, D_MODEL], BF16, kind="ExternalOutput")

    Exp = mybir.ActivationFunctionType.Exp
    add = mybir.AluOpType.add
    mult = mybir.AluOpType.mult

    n_combos = 16

    with tile.TileContext(nc) as tc:
        with (
            tc.tile_pool(name="const", bufs=1) as const_pool,
            tc.tile_pool(name="qt", bufs=3) as qt_pool,
            tc.tile_pool(name="expT", bufs=2) as expT_pool,
            tc.tile_pool(name="chain", bufs=4) as chain_pool,
            tc.tile_pool(name="acc", bufs=2) as acc_pool,
            tc.tile_pool(name="sums", bufs=2) as sums_pool,
            tc.tile_pool(name="rb", bufs=2) as rb_pool,
            tc.tile_pool(name="attnT", bufs=8) as attnT_pool,
            tc.tile_pool(name="orow", bufs=2) as orow_pool,
            tc.tile_pool(name="qk_ps", bufs=2, space="PSUM") as qk_psum,
            tc.tile_pool(name="pv_ps", bufs=2, space="PSUM") as pv_psum,
            tc.tile_pool(name="po_ps", bufs=2, space="PSUM") as po_psum,
        ):
            # startup: first QK needs kT chunk 0 + qT(0). Halves triggered
            # from two engines so descriptor writes and transfers overlap.
            kT_chunks = []
            for c in range(4):
                kc = const_pool.tile([P, 512], F32R, tag=f"kT{c}")
                kT_chunks.append(kc)
            qt0 = qt_pool.tile([P, 512], F32R, tag="qT")
            nc.sync.dma_start(qt0[:, 0:256], qT_d[:, 0, ds(0, 256)].bitcast(F32R))
            nc.scalar.dma_start(qt0[:, 256:512], qT_d[:, 0, ds(256, 256)].bitcast(F32R))
            nc.sync.dma_start(kT_chunks[0][:, 0:256], kT_d[:, ds(0, 256)].bitcast(F32R))
            nc.scalar.dma_start(kT_chunks[0][:, 256:512], kT_d[:, ds(256, 256)].bitcast(F32R))
            nc.sync.dma_start(kT_chunks[1][:], kT_d[:, ts(1, 512)].bitcast(F32R))
            nc.scalar.dma_start(kT_chunks[2][:], kT_d[:, ts(2, 512)].bitcast(F32R))
            nc.scalar.dma_start(kT_chunks[3][:], kT_d[:, ts(3, 512)].bitcast(F32R))
            # v (bf16, 512KB) on the gpsimd queue; wT (4MB) behind it --
            # first use of wT is the first proj slice ~45us in.
            v_sb = const_pool.tile([P, NT, P], BF16, tag="v")
            nc.gpsimd.dma_start(v_sb[:], v_d.rearrange("(n p) d -> p n d", p=P))
            wT_sb = const_pool.tile([P, HEADS_PER_GROUP, D_MODEL], F32R, tag="wT")
            nc.gpsimd.dma_start(
                wT_sb[:], wT_d.rearrange("(n p) o -> p n o", p=P).bitcast(F32R)
            )

            qts = {0: qt0}
            ets = {}      # k -> exp tile [P, 16, 512] bf16
            attnT = {}
            pv_tiles = {}
            sums_tiles = {}

            def emit_po(jp, stp, ob, orow_tile, copy_engine):
                po = po_psum.tile([P, 512], F32, tag="po")
                for h in range(HEADS_PER_GROUP):
                    nc.tensor.matmul(
                        po[:],
                        attnT[4 * jp + h][:, ts(stp, P)],
                        wT_sb[:, h, ts(ob, 512)],
                        start=(h == 0), stop=(h == HEADS_PER_GROUP - 1),
                    )
                if copy_engine == "dve":
                    nc.vector.tensor_copy(orow_tile[:, ts(ob, 512)], po[:])
                else:
                    nc.scalar.copy(orow_tile[:, ts(ob, 512)], po[:])

            for k in range(n_combos + 5):
                kq = k + 1
                if kq < n_combos and kq not in qts:
                    jq, hq = divmod(kq, HEADS_PER_GROUP)
                    qt = qt_pool.tile([P, 512], F32R, tag="qT")
                    nc.sync.dma_start(qt[:], qT_d[:, hq, ts(jq, 512)].bitcast(F32R))
                    qts[kq] = qt

                do_qk = k < n_combos
                do_pv = 1 <= k <= n_combos
                do_proj = 5 <= k <= n_combos + 4
                if do_proj:
                    jp, stp = divmod(k - 5, 4)
                    jp_orow = orow_pool.tile([P, D_MODEL], BF16, tag="orow")

                if do_pv:
                    pv_ps = pv_psum.tile([P, 512], F32, tag="pv")
                    pv_tiles[k - 1] = pv_ps

                if do_qk:
                    et = expT_pool.tile([P, NT, 512], BF16, tag="expT")
                    ets[k] = et

                chain_prev = None
                for cc in range(8):
                    if do_qk:
                        ps = qk_psum.tile([P, 2, 512], F32, tag="qk")
                        for u in range(2):
                            tt = 2 * cc + u
                            nc.tensor.matmul(
                                ps[:, u, :],
                                kT_chunks[tt // 4][:, ts(tt % 4, P)],
                                qts[k][:],
                                start=True, stop=True,
                            )
                    if do_pv:
                        for u in range(2):
                            tt = 2 * cc + u
                            nc.tensor.matmul(
                                pv_ps[:],
                                v_sb[:, tt, :],
                                ets[k - 1][:, tt, :],
                                start=(tt == 0), stop=(tt == NT - 1),
                            )
                    # proj po-blocks occupy the back half of the iteration so
                    # the QK chunks (which gate exp, which gates everything)
                    # all land in the front 6us
                    if do_proj and cc >= 4:
                        ob = cc - 4
                        if do_qk:
                            eng = "dve" if ob < 3 else "act"
                        else:
                            eng = "dve" if ob % 2 == 0 else "act"
                        emit_po(jp, stp, ob, jp_orow, eng)
                    if do_qk:
                        nc.scalar.activation(
                            et[:, ds(2 * cc, 2), :], ps[:], Exp, scale=SCALE
                        )
                        # incremental denominator chain: one bf16 add per
                        # chunk, so the chain finishes ~1 op after the last exp
                        if cc == 1:
                            r = chain_pool.tile([P, 2, 512], BF16, tag="chain")
                            nc.vector.tensor_tensor(
                                r[:], et[:, 0:2, :], et[:, 2:4, :], add
                            )
                            chain_prev = r
                        elif cc >= 2:
                            r = chain_pool.tile([P, 2, 512], BF16, tag="chain")
                            nc.vector.tensor_tensor(
                                r[:], chain_prev[:], et[:, ds(2 * cc, 2), :], add
                            )
                            chain_prev = r

                if do_proj:
                    nc.sync.dma_start(
                        out_d[ds(jp * 512 + stp * P, P), :], jp_orow[:]
                    )

                if do_qk:
                    acc = acc_pool.tile([P, 512], F32, tag="acc")
                    nc.vector.tensor_tensor(
                        acc[:], chain_prev[:, 0, :], chain_prev[:, 1, :], add
                    )
                    sums_bc = sums_pool.tile([P, 512], F32, tag="sums")
                    nc.gpsimd.partition_all_reduce(
                        sums_bc[:], acc[:], channels=P,
                        reduce_op=bass_isa.ReduceOp.add,
                    )
                    sums_tiles[k] = sums_bc

                if do_pv:
                    # normalize combo k-1 at the very end of the DVE queue:
                    # its all-reduce ran during this iteration and its
                    # consumers (proj of group (k-1)//4) are >=1 iteration out
                    rb = rb_pool.tile([P, 512], F32, tag="rb")
                    nc.vector.reciprocal_approx_fast(rb[:], sums_tiles[k - 1][:])
                    at = attnT_pool.tile([P, 512], F32R, tag="attnT")
                    nc.vector.tensor_tensor(
                        at[:], pv_tiles[k - 1][:], rb[:], mult
                    )
                    attnT[k - 1] = at

    nc.compile()
    return nc


def _get_nc():
    global _COMPILED
    if _COMPILED is None:
        _COMPILED = _build()
    return _COMPILED


def _shard_inputs(q, k, v, Wc):
    in_maps = []
    for c in range(8):
        b, g = divmod(c, 4)
        qT = np.ascontiguousarray(
            q[b][:, g * 512:(g + 1) * 512].reshape(S, HEADS_PER_GROUP, P).transpose(2, 1, 0)
        )
        kT = np.ascontiguousarray(k[b][:, g * P:(g + 1) * P].T)
        vv = np.ascontiguousarray(v[b][:, g * P:(g + 1) * P]).astype(ml_dtypes.bfloat16)
        wT = np.ascontiguousarray(Wc[:, g * 512:(g + 1) * 512].T)
        in_maps.append({"qT": qT, "kT": kT, "v": vv, "wT": wT})
    return in_maps


def _run(inputs, trace=False):
    q = np.asarray(inputs["q"], dtype=np.float32)
    k = np.asarray(inputs["k"], dtype=np.float32)
    v = np.asarray(inputs["v"], dtype=np.float32)
    Wc = np.asarray(inputs["Wc"], dtype=np.float32)
    bc = np.asarray(inputs["bc"], dtype=np.float32)

    nc = _get_nc()
    in_maps = _shard_inputs(q, k, v, Wc)
    res = run_bass_kernel_spmd(nc, in_maps, list(range(8)), trace=trace)

    out = np.empty((B, S, D_MODEL), dtype=np.float32)
    for b in range(B):
        acc = res.results[4 * b]["out"].astype(np.float32)
        for g in range(1, 4):
            acc = acc + res.results[4 * b + g]["out"].astype(np.float32)
        acc += bc.reshape(1, D_MODEL)
        out[b] = acc
    return out, res


def kernel(**inputs):
    out, _ = _run(inputs, trace=False)
    return out


# revision 9
# speedup vs baseline: 1.1008x; 1.0877x over previous
"""GQA (B=2, S=2048, d_model=2048, 16 Q heads / 4 KV groups) + output projection.

Sharding: 8 cores, core c <-> (b = c//4, g = c%4). Each core computes full
attention for the 4 query heads of KV group g of batch b, then multiplies its
512-feature slice of the concatenated head outputs with the matching 512 rows
of Wc^T, producing a partial [S, d_model] projection (bf16). Host sums the 4
partials per batch element and adds the bias.

On-core layout: everything transposed.
  scoresT[t, s] = kT.T @ qT            (lhsT = kT tile [d,128t], rhs = qT [d,512s])
  expT = exp(scoresT / sqrt(128))      (ACT, fused scale, bf16 out; no max
                                        subtraction: scores ~ N(0,1))
  sums  = partition_all_reduce(H-tree(expT))
                                       (DVE bf16 4-op halving tree + one GPSIMD
                                        partition all-reduce -- NO PE matmuls
                                        for the softmax denominator; this freed
                                        25% of the baseline's PE work)
  uT[hd, s]   = v.T @ expT             (PE, bf16 operands, accumulated over 16 t)
  attnT = uT * recip(sums)             (DVE recip + mult, deferred to the END of
                                        the next iteration's DVE queue so the
                                        ~3.5us Pool all-reduce latency never
                                        head-of-line blocks copies the PE needs)
  out[s, o]   = attnT.T @ wT           (PE fp32r, contraction over 512 features;
                                        bias added on host)

Scheduling: per-iteration k the PE stream interleaves, per 2-t-tile step,
QK(k) pairs + PV(k-1) pairs + 2 proj matmuls of the slice (jp=(k-5)//4,
st=(k-5)%4). Spreading the projection into the following group's combos keeps
the PE stream dense; the per-combo softmax epilogue rides on DVE/Pool.
"""

import math
import sys

sys.path.insert(0, "/opt/trn_rl_repo")

import ml_dtypes
import numpy as np

import concourse.bacc as bacc
import concourse.bass as bass
import concourse.bass_isa as bass_isa
import concourse.mybir as mybir
import concourse.tile as tile
from concourse.bass import ds, ts
from concourse.bass_utils import run_bass_kernel_spmd

F32 = mybir.dt.float32
F32R = mybir.dt.float32r
BF16 = mybir.dt.bfloat16

B = 2
S = 2048
D_MODEL = 2048
N_GROUPS = 4
HEADS_PER_GROUP = 4
HEAD_DIM = 128
P = 128
NT = S // P          # 16 t tiles
SCALE = 1.0 / math.sqrt(HEAD_DIM)

_COMPILED = None


def _build():
    nc = bacc.Bacc(None, target_bir_lowering=False)

    qT_d = nc.dram_tensor("qT", [P, HEADS_PER_GROUP, S], F32, kind="ExternalInput")
    kT_d = nc.dram_tensor("kT", [P, S], F32, kind="ExternalInput")
    v_d = nc.dram_tensor("v", [S, P], BF16, kind="ExternalInput")
    wT_d = nc.dram_tensor("wT", [HEADS_PER_GROUP * P, D_MODEL], F32, kind="ExternalInput")
    out_d = nc.dram_tensor("out", [S, D_MODEL], BF16, kind="ExternalOutput")

    Exp = mybir.ActivationFunctionType.Exp
    add = mybir.AluOpType.add
    mult = mybir.AluOpType.mult

    n_combos = 16

    with tile.TileContext(nc) as tc:
        with (
            tc.tile_pool(name="const", bufs=1) as const_pool,
            tc.tile_pool(name="qt", bufs=3) as qt_pool,
            tc.tile_pool(name="expT", bufs=2) as expT_pool,
            tc.tile_pool(name="tr8", bufs=2) as tr8_pool,
            tc.tile_pool(name="tr4", bufs=2) as tr4_pool,
            tc.tile_pool(name="tr2", bufs=2) as tr2_pool,
            tc.tile_pool(name="acc", bufs=2) as acc_pool,
            tc.tile_pool(name="sums", bufs=2) as sums_pool,
            tc.tile_pool(name="rb", bufs=2) as rb_pool,
            tc.tile_pool(name="attnT", bufs=8) as attnT_pool,
            tc.tile_pool(name="orow", bufs=2) as orow_pool,
            tc.tile_pool(name="qk_ps", bufs=2, space="PSUM") as qk_psum,
            tc.tile_pool(name="pv_ps", bufs=2, space="PSUM") as pv_psum,
            tc.tile_pool(name="po_ps", bufs=2, space="PSUM") as po_psum,
        ):
            # startup: first QK needs kT chunk 0 + qT(0); trigger them first
            # on separate engine queues so the transfers overlap. kT chunk 3
            # rides the gpsimd queue ahead of v/wT.
            kT_chunks = []
            for c in range(4):
                kc = const_pool.tile([P, 512], F32R, tag=f"kT{c}")
                kT_chunks.append(kc)
            nc.sync.dma_start(kT_chunks[0][:], kT_d[:, ts(0, 512)].bitcast(F32R))
            qt0 = qt_pool.tile([P, 512], F32R, tag="qT")
            nc.scalar.dma_start(qt0[:], qT_d[:, 0, ts(0, 512)].bitcast(F32R))
            nc.sync.dma_start(kT_chunks[1][:], kT_d[:, ts(1, 512)].bitcast(F32R))
            nc.scalar.dma_start(kT_chunks[2][:], kT_d[:, ts(2, 512)].bitcast(F32R))
            nc.gpsimd.dma_start(kT_chunks[3][:], kT_d[:, ts(3, 512)].bitcast(F32R))
            # v (bf16, 512KB) then wT (4MB) on the gpsimd queue -- first use
            # of wT is the first proj slice ~45us in.
            v_sb = const_pool.tile([P, NT, P], BF16, tag="v")
            nc.gpsimd.dma_start(v_sb[:], v_d.rearrange("(n p) d -> p n d", p=P))
            wT_sb = const_pool.tile([P, HEADS_PER_GROUP, D_MODEL], F32R, tag="wT")
            nc.gpsimd.dma_start(
                wT_sb[:], wT_d.rearrange("(n p) o -> p n o", p=P).bitcast(F32R)
            )

            qts = {0: qt0}
            ets = {}      # k -> exp tile [P, 16, 512] bf16
            attnT = {}
            pv_tiles = {}
            sums_tiles = {}

            for k in range(n_combos + 5):
                # prefetch next combo's qT one iteration ahead
                kq = k + 1
                if kq < n_combos and kq not in qts:
                    jq, hq = divmod(kq, HEADS_PER_GROUP)
                    qt = qt_pool.tile([P, 512], F32R, tag="qT")
                    nc.sync.dma_start(qt[:], qT_d[:, hq, ts(jq, 512)].bitcast(F32R))
                    qts[kq] = qt

                do_qk = k < n_combos
                do_pv = 1 <= k <= n_combos
                # proj slice: group jp, row-block st, fed by combos of group jp+1
                do_proj = 5 <= k <= n_combos + 4
                if do_proj:
                    jp, stp = divmod(k - 5, 4)
                    jp_orow = orow_pool.tile([P, D_MODEL], BF16, tag="orow")

                if do_pv:
                    pv_ps = pv_psum.tile([P, 512], F32, tag="pv")
                    pv_tiles[k - 1] = pv_ps

                if do_qk:
                    et = expT_pool.tile([P, NT, 512], BF16, tag="expT")
                    ets[k] = et

                for cc in range(8):
                    if do_qk:
                        ps = qk_psum.tile([P, 2, 512], F32, tag="qk")
                        for u in range(2):
                            tt = 2 * cc + u
                            nc.tensor.matmul(
                                ps[:, u, :],
                                kT_chunks[tt // 4][:, ts(tt % 4, P)],
                                qts[k][:],
                                start=True, stop=True,
                            )
                    if do_pv:
                        for u in range(2):
                            tt = 2 * cc + u
                            nc.tensor.matmul(
                                pv_ps[:],
                                v_sb[:, tt, :],
                                ets[k - 1][:, tt, :],
                                start=(tt == 0), stop=(tt == NT - 1),
                            )
                    if do_proj:
                        ob, uh = divmod(cc, 2)
                        if uh == 0:
                            po = po_psum.tile([P, 512], F32, tag="po")
                        for h in (0, 1) if uh == 0 else (2, 3):
                            nc.tensor.matmul(
                                po[:],
                                attnT[4 * jp + h][:, ts(stp, P)],
                                wT_sb[:, h, ts(ob, 512)],
                                start=(h == 0), stop=(h == HEADS_PER_GROUP - 1),
                            )
                        if uh == 1:
                            if do_qk and ob == 3:
                                # ACT has slack after its 8 exps; keep the DVE
                                # queue short for the tree + normalize
                                nc.scalar.copy(jp_orow[:, ts(ob, 512)], po[:])
                            elif do_qk or ob % 2 == 0:
                                nc.vector.tensor_copy(jp_orow[:, ts(ob, 512)], po[:])
                            else:
                                nc.scalar.copy(jp_orow[:, ts(ob, 512)], po[:])
                    if do_qk:
                        nc.scalar.activation(
                            et[:, ds(2 * cc, 2), :], ps[:], Exp, scale=SCALE
                        )

                if do_proj:
                    nc.sync.dma_start(
                        out_d[ds(jp * 512 + stp * P, P), :], jp_orow[:]
                    )

                if do_qk:
                    # softmax denominator: 4-op halving tree (bf16, 2x DVE),
                    # then cross-partition all-reduce on the Pool engine
                    t8 = tr8_pool.tile([P, 8, 512], BF16, tag="tr8")
                    nc.vector.tensor_tensor(
                        t8[:], et[:, 0:8, :], et[:, 8:16, :], add
                    )
                    t4 = tr4_pool.tile([P, 4, 512], BF16, tag="tr4")
                    nc.vector.tensor_tensor(t4[:], t8[:, 0:4, :], t8[:, 4:8, :], add)
                    t2 = tr2_pool.tile([P, 2, 512], BF16, tag="tr2")
                    nc.vector.tensor_tensor(t2[:], t4[:, 0:2, :], t4[:, 2:4, :], add)
                    acc = acc_pool.tile([P, 512], F32, tag="acc")
                    nc.vector.tensor_tensor(acc[:], t2[:, 0, :], t2[:, 1, :], add)
                    sums_bc = sums_pool.tile([P, 512], F32, tag="sums")
                    nc.gpsimd.partition_all_reduce(
                        sums_bc[:], acc[:], channels=P,
                        reduce_op=bass_isa.ReduceOp.add,
                    )
                    sums_tiles[k] = sums_bc

                if do_pv:
                    # normalize combo k-1 now: the Pool all-reduce for k-1 ran
                    # during this iteration, so this never blocks the DVE queue
                    rb = rb_pool.tile([P, 512], F32, tag="rb")
                    nc.vector.reciprocal_approx_fast(rb[:], sums_tiles[k - 1][:])
                    at = attnT_pool.tile([P, 512], F32R, tag="attnT")
                    nc.vector.tensor_tensor(
                        at[:], pv_tiles[k - 1][:], rb[:], mult
                    )
                    attnT[k - 1] = at

    nc.compile()
    return nc


def _get_nc():
    global _COMPILED
    if _COMPILED is None:
        _COMPILED = _build()
    return _COMPILED


def _shard_inputs(q, k, v, Wc):
    in_maps = []
    for c in range(8):
        b, g = divmod(c, 4)
        qT = np.ascontiguousarray(
            q[b][:, g * 512:(g + 1) * 512].reshape(S, HEADS_PER_GROUP, P).transpose(2, 1, 0)
        )
        kT = np.ascontiguousarray(k[b][:, g * P:(g + 1) * P].T)
        vv = np.ascontiguousarray(v[b][:, g * P:(g + 1) * P]).astype(ml_dtypes.bfloat16)
        wT = np.ascontiguousarray(Wc[:, g * 512:(g + 1) * 512].T)
        in_maps.append({"qT": qT, "kT": kT, "v": vv, "wT": wT})
    return in_maps


def _run(inputs, trace=False):
    q = np.asarray(inputs["q"], dtype=np.float32)
    k = np.asarray(inputs["k"], dtype=np.float32)
    v = np.asarray(inputs["v"], dtype=np.float32)
    Wc = np.asarray(inputs["Wc"], dtype=np.float32)
    bc = np.asarray(inputs["bc"], dtype=np.float32)

    nc = _get_nc()
    in_maps = _shard_inputs(q, k, v, Wc)
    res = run_bass_kernel_spmd(nc, in_maps, list(range(8)), trace=trace)

    out = np.empty((B, S, D_MODEL), dtype=np.float32)
    for b in range(B):
        acc = res.results[4 * b]["out"].astype(np.float32)
        for g in range(1, 4):
            acc = acc + res.results[4 * b + g]["out"].astype(np.float32)
        acc += bc.reshape(1, D_MODEL)
        out[b] = acc
    return out, res


def kernel(**inputs):
    out, _ = _run(inputs, trace=False)
    return out
